# revision 16
# baseline (speedup 1.0000x reference)
"""Trainium2 Bass kernel for nn_Model_39676907882504.

Math: qk = (q @ k^T)/8 has shape [1,2048,1,1]; after the transposes it is
[2048,1,1,1], and softmax over the trailing size-1 axis is exactly 1.0
regardless of qk (exp(x-max)/sum == 1/1 bit-exactly). The final matmul
[S,Q,B,Q] @ [B,S,Q,D] with attn_weight == 1 therefore reduces to
broadcasting `value` across a new leading dim:

    output[i, j, 0, :] = value[0, j, 0, :]   for all i in [0, 2048)

i.e. a 512KB -> 1GiB broadcast copy.  Pure memory-regime kernel.

Sharding (per the hint): leading output dim (2048 rows) split across the
8 cores, 256 rows/core (= 4096 sub-rows of 32KB; sub-row d holds value
dchunk d%8); value replicated.

Per-core plan (trace-derived):

  * One 4MiB load of a host-pre-tiled value [128, 8192] (partition q =
    dchunk q%8).
  * TWO giant store instructions (one per HW-DGE queue: sync=SP,
    scalar=ACT), each 64MiB = 2048 descriptors of 32KB.  The SBUF side
    re-reads the same 4MiB via a stride-0 middle dim [128, 16, 8192];
    the DRAM side is the transposed view out.rearrange("(c q) e ->
    q c e") so descriptor (q, c) lands at sub-row q + 128*c, whose
    required content is dchunk q%8.
  * Descriptors split across the 16 SDMA engines contiguously along
    the outer (partition) dim -> 8 partitions x 16 reps = 128 descs of
    32KB per engine per instruction, each moved at the ~27 GB/s
    per-engine line rate (~1216ns/descriptor) -> engine-bound at ~425
    GB/s aggregate.
  * Variants measured and rejected: >2 store instructions (SDMA engine
    15 pays a ~2.7us completion stall per instruction); any non-16-way
    store (outer dim 120/124/8 -> uneven engine fan-out or descriptor
    starvation, +40..140us); a 64-partition tile at partitions 32-95
    (halves the load, but regressed under HBM-domain contention).

Engine budget: 8 load + 256 store descriptors x ~1.216us = ~321us plus
~13us fixed NEFF entry/exit and a load/store barrier -> ~346us measured
(can reach ~390us when the paired NeuronCore's HBM-domain traffic or
the intermittent SDMA-engine-15 degradation is present).
"""

import sys

for _p in ("/opt/trn_rl_repo",):
    if _p not in sys.path:
        sys.path.insert(0, _p)

import numpy as np

import concourse.bass as bass
import concourse.mybir as mybir
from concourse.bass_utils import run_bass_kernel_spmd

S = 2048
D = 64
N_CORES = 8
ROWS_PER_CORE = S // N_CORES          # 256 output rows/core, 512KB each
F = 16384                             # f32 per 64KB dchunk; value = 8 dchunks
SUBROWS = ROWS_PER_CORE * 8           # 2048 64KB sub-rows per shard
NREP = SUBROWS // 128                 # 32 broadcast reps of the 128-part tile

TRACE = False          # test.py flips this to profile
TRACE_KWARGS = {}
LAST_RESULT = None     # BassKernelResults of the last run (for test.py)


def build_program():
    nc = bass.Bass()
    val = nc.declare_dram_parameter("value", [128, F], mybir.dt.float32,
                                    isOutput=False)
    out = nc.declare_dram_parameter("out", [SUBROWS, F], mybir.dt.float32,
                                    isOutput=True)

    vtile = nc.alloc_sbuf_tensor("vtile", [128, F], mybir.dt.float32)

    # [q, c, e]: sub-row q + 128*c <- vtile partition q (dchunk q%8), so
    # every sub-row d gets dchunk d%8.  Split the c axis across queues.
    out_qce = out[:, :].rearrange("(c q) e -> q c e", q=128)
    half = NREP // 2

    def in_bcast(reps):
        return vtile[:, :].unsqueeze(1).broadcast_to((128, reps, F))

    with nc.Block() as block, \
         nc.semaphore("lsem") as lsem, \
         nc.semaphore("ssem") as ssem:

        @block.sync
        def _(sync):
            sync.dma_start(out=vtile[:, :], in_=val[:, :]).then_inc(lsem, 16)
            # No wait before the sync-queue store: each SDMA engine
            # processes its ring in FIFO order, and engine k's first
            # store descriptor re-reads the partition its own first
            # load descriptor wrote ~8 descriptors (~10us) earlier.
            sync.dma_start(out=out_qce[:, 0:half, :],
                           in_=in_bcast(half)).then_inc(ssem, 16)
            sync.wait_ge(ssem, 32)

        @block.scalar
        def _(scalar):
            scalar.wait_ge(lsem, 16)
            scalar.dma_start(out=out_qce[:, half:NREP, :],
                             in_=in_bcast(NREP - half)).then_inc(ssem, 16)
            scalar.wait_ge(ssem, 32)

    return nc


def kernel(query=None, key=None, value=None, attn_mask=None, **_ignored):
    global LAST_RESULT
    value = np.ascontiguousarray(np.asarray(value, dtype=np.float32))
    vflat = value.reshape(8, F)                       # 8 dchunks of 64KB
    vexp = np.ascontiguousarray(np.tile(vflat, (16, 1)))  # [128, F]

    nc = build_program()
    core_ids = list(range(N_CORES))
    in_maps = [{"value": vexp} for _ in core_ids]
    res = run_bass_kernel_spmd(nc, in_maps, core_ids, trace=TRACE,
                               **TRACE_KWARGS)
    LAST_RESULT = res

    # Core i supplies output rows [i*256, (i+1)*256).
    shards = [res.results[i]["out"].reshape(ROWS_PER_CORE, S, 1, D)
              for i in range(N_CORES)]
    return np.concatenate(shards, axis=0)


# revision 17
# speedup vs baseline: 1.2171x; 1.2171x over previous
"""Trainium2 Bass kernel for nn_Model_39676907882504.

Math: qk = (q @ k^T)/8 has shape [1,2048,1,1]; after the transposes it is
[2048,1,1,1], and softmax over the trailing size-1 axis is exactly 1.0
regardless of qk (exp(x-max)/sum == 1/1 bit-exactly). The final matmul
[S,Q,B,Q] @ [B,S,Q,D] with attn_weight == 1 therefore reduces to
broadcasting `value` across a new leading dim:

    output[i, j, 0, :] = value[0, j, 0, :]   for all i in [0, 2048)

i.e. a 512KB -> 1GiB broadcast copy.  Pure memory-regime kernel.

Sharding (per the hint): leading output dim (2048 rows) split across the
8 cores, 256 rows/core (= 4096 sub-rows of 32KB; sub-row d holds value
chunk d%16); value replicated.

Per-core plan (trace-derived):

  * One 4MiB load of a host-pre-tiled value [128, 8192] (partition q =
    chunk q%16).
  * TWO giant store instructions (one per HW-DGE queue: sync=SP,
    scalar=ACT), each 64MiB = 2048 descriptors of 32KB.  The SBUF side
    re-reads the same 4MiB via a stride-0 middle dim [128, 16, 8192];
    the DRAM side is the transposed view out.rearrange("(c q) e ->
    q c e") so descriptor (q, c) lands at sub-row q + 128*c, whose
    required content is chunk q%16.
  * Descriptors split across the 16 SDMA engines contiguously along
    the outer (partition) dim -> 8 partitions x 16 reps = 128 descs of
    32KB per engine per instruction, each moved at the ~27 GB/s
    per-engine line rate (~1216ns/descriptor) -> engine-bound at ~425
    GB/s aggregate.
  * Variants measured and rejected: >2 store instructions (SDMA engine
    15 pays a ~2.7us completion stall per instruction); any non-16-way
    store (outer dim 120/124/8 -> uneven engine fan-out or descriptor
    starvation, +40..140us); a 64-partition tile at partitions 32-95
    (halves the load, but regressed under HBM-domain contention).

Engine budget: 8 load + 256 store descriptors x ~1.216us = ~321us plus
~13us fixed NEFF entry/exit and a load/store barrier -> ~346us measured
(can reach ~390us when the paired NeuronCore's HBM-domain traffic or
the intermittent SDMA-engine-15 degradation is present).
"""

import sys

for _p in ("/opt/trn_rl_repo",):
    if _p not in sys.path:
        sys.path.insert(0, _p)

import numpy as np

import concourse.bass as bass
import concourse.mybir as mybir
from concourse.bass_utils import run_bass_kernel_spmd

S = 2048
D = 64
N_CORES = 8
ROWS_PER_CORE = S // N_CORES          # 256 output rows/core, 512KB each
F = 8192                              # f32 per 32KB chunk; value = 16 chunks
SUBROWS = ROWS_PER_CORE * 16          # 4096 32KB sub-rows per shard
NREP = SUBROWS // 128                 # 32 broadcast reps of the 128-part tile

TRACE = False          # test.py flips this to profile
TRACE_KWARGS = {}
LAST_RESULT = None     # BassKernelResults of the last run (for test.py)


def build_program():
    nc = bass.Bass()
    val = nc.declare_dram_parameter("value", [128, F], mybir.dt.float32,
                                    isOutput=False)
    out = nc.declare_dram_parameter("out", [SUBROWS, F], mybir.dt.float32,
                                    isOutput=True)

    vtile = nc.alloc_sbuf_tensor("vtile", [128, F], mybir.dt.float32)

    # [q, c, e]: sub-row q + 128*c <- vtile partition q (chunk q%16), so
    # every sub-row d gets chunk d%16.  Split the c axis across queues.
    out_qce = out[:, :].rearrange("(c q) e -> q c e", q=128)
    half = NREP // 2

    def in_bcast(reps):
        return vtile[:, :].unsqueeze(1).broadcast_to((128, reps, F))

    with nc.Block() as block, \
         nc.semaphore("lsem") as lsem, \
         nc.semaphore("ssem") as ssem:

        @block.sync
        def _(sync):
            sync.dma_start(out=vtile[:, :], in_=val[:, :]).then_inc(lsem, 16)
            # No wait before the sync-queue store: each SDMA engine
            # processes its ring in FIFO order, and engine k's first
            # store descriptor re-reads the partition its own first
            # load descriptor wrote ~8 descriptors (~10us) earlier.
            sync.dma_start(out=out_qce[:, 0:half, :],
                           in_=in_bcast(half)).then_inc(ssem, 16)
            sync.wait_ge(ssem, 32)

        @block.scalar
        def _(scalar):
            scalar.wait_ge(lsem, 16)
            scalar.dma_start(out=out_qce[:, half:NREP, :],
                             in_=in_bcast(NREP - half)).then_inc(ssem, 16)
            scalar.wait_ge(ssem, 32)

    return nc


def kernel(query=None, key=None, value=None, attn_mask=None, **_ignored):
    global LAST_RESULT
    value = np.ascontiguousarray(np.asarray(value, dtype=np.float32))
    vflat = value.reshape(16, F)                      # 16 chunks of 32KB
    vexp = np.ascontiguousarray(np.tile(vflat, (8, 1)))   # [128, F]

    nc = build_program()
    core_ids = list(range(N_CORES))
    in_maps = [{"value": vexp} for _ in core_ids]
    res = run_bass_kernel_spmd(nc, in_maps, core_ids, trace=TRACE,
                               **TRACE_KWARGS)
    LAST_RESULT = res

    # Core i supplies output rows [i*256, (i+1)*256).
    shards = [res.results[i]["out"].reshape(ROWS_PER_CORE, S, 1, D)
              for i in range(N_CORES)]
    return np.concatenate(shards, axis=0)
